# revision 35
# baseline (speedup 1.0000x reference)
"""LSEP loss kernel for Trainium2, data-parallel over 8 NeuronCores.

Math per element i (B=1e6, C=10):
  q[c]  = T[i, bayes[i], c]
  s_neg = sum_c (partial[i,c]==0) * exp(q[c])
  s_pos = sum_c (partial[i,c]==1) * exp(-q[c])
  loss  = mean_i log1p(s_neg * s_pos)

Strategy: the loss is a mean, so elements can be freely permuted. Host-side
we bucket elements by bayes value and give every core a static layout of
10 sections (one per bayes value v), each 128 partitions x 100 slots, padded
with null elements (T=0, partial=1 -> contributes ~2e-7 rel, negligible).
Row selection then needs no gather: section v reads T columns [v*10, v*10+10)
through a static strided access pattern, so no compute engine touches the 90
unused T values -- they only flow through DMA, which is the intended
memory-bound term. T is staged as fp8(e3m4) rows of 100 bytes.

The partial mask is folded ADDITIVELY: the host writes m = -15.5*p as fp8
(both values exact; 15.5 is the e3m4 max normal) into the DEAD row
(v+1)%10 of each element's T block -- that row is never read for a
section-v element, so the mask rides along in bytes that were already
being transferred. The device computes u = q + m in one gpsimd add
(bf16 out, exact for p=0 since q is fp8).
Then  s_neg = sum_c exp(u)        (masked terms are scaled by e^-15.5)
      s_pos = sum_c exp(-u - 15.5)  (ACT's free scale/bias: func(s*x+b))
so each section is exactly: 1 gpsimd add, 2 ACT exps (flat bf16), 2 DVE
innermost-reduces, 1 small product. No mask multiplies, no DVE two-port ops
(which lock gpsimd out of SBUF), minimal SBUF traffic: under concurrent DMA
streaming every engine's effective rate drops ~2x from port contention, so
the op mix matters more than isolated op speed. Verified numerics vs the
f32 reference: rel err 2.6e-4 (gate 2e-2). Epilogue: log1p, row-sum, one
[128,1] output per core; host sums across cores/partitions, divides by B.
"""

from contextlib import ExitStack

import numpy as np

import concourse.bacc as bacc
import concourse.mybir as mybir
import concourse.tile as tile
from concourse.bass_utils import run_bass_kernel_spmd

f32 = mybir.dt.float32
bf16 = mybir.dt.bfloat16
f8 = mybir.dt.float8e3
Alu = mybir.AluOpType
Act = mybir.ActivationFunctionType
Axis = mybir.AxisListType

B = 1_000_000
C = 10
CC = C * C
NCORES = 8
V = C  # bayes values / sections
P = 128
NJ = 100  # slots per partition per section
S_V = P * NJ  # 12800 slots per (core, section)
S_CORE = V * S_V  # 128000 slots per core
BIG = 15.5  # e3m4 max normal; exp(-BIG) ~ 1.9e-7 suppresses masked terms
assert NCORES * S_V >= B // V + 8 * 300  # ~8 sigma headroom per bucket


def build_core_program(nc):
    T_d = nc.dram_tensor("t_in", [S_CORE, CC], f8, kind="ExternalInput").ap()
    out_d = nc.dram_tensor("sum_out", [1, 1], f32, kind="ExternalOutput").ap()

    T_view = T_d.rearrange("(v p j) c -> v p (j c)", v=V, p=P, j=NJ)

    # full tiles up front (transfer > descriptor-gen keeps the DMA ring fed;
    # small leading tiles starve it), split tiles at the end to shorten the
    # serial tail chain after the last DMA lands
    q = NJ // 4
    subtiles = [(v, 0, NJ) for v in range(V - 1)]
    subtiles += [(V - 1, 0, 2 * q), (V - 1, 2 * q, q), (V - 1, 3 * q, q)]

    with tile.TileContext(nc) as tc, ExitStack() as ctx:
        big = ctx.enter_context(tc.tile_pool(name="big", bufs=4))
        work = ctx.enter_context(tc.tile_pool(name="work", bufs=3))
        acc = ctx.enter_context(tc.tile_pool(name="acc", bufs=1))

        prodbuf = acc.tile([P, V * NJ], f32)
        bigbias = acc.tile([P, 1], f32)
        nc.vector.memset(bigbias[:], -BIG)

        for v, j0, nj in subtiles:
            t = big.tile([P, nj * CC], f8, tag="t")
            nc.sync.dma_start(t[:], T_view[v, :, j0 * CC : (j0 + nj) * CC])
            tv = t[:].rearrange("p (j c) -> p j c", j=nj)
            tsel = tv[:, :, v * C : (v + 1) * C]  # [P, nj, 10]: T row v
            # mask -BIG*p lives in the dead row (v+1)%V of each element's
            # T block (never read for a section-v element)
            w = (v + 1) % V
            pm = tv[:, :, w * C : (w + 1) * C]
            n = nj * C

            # u = q - BIG*p in one gpsimd add (bf16; exact where p=0)
            u = work.tile([P, n], bf16, tag="u")
            nc.gpsimd.tensor_tensor(
                u[:].rearrange("p (j c) -> p j c", j=nj), tsel, pm, op=Alu.add
            )

            # both exps land c-major in one tile ([c, sign, j] planes), so the
            # c-sum runs as a pairwise tree of flat bf16 TT adds (measured
            # ~2x faster per element than TENSOR_REDUCE, which is 1x-capped)
            jj = 2 * nj
            e2 = work.tile([P, C * jj], bf16, tag="e2")
            e2t = e2[:].rearrange("p (c jj) -> p jj c", c=C)  # strided view
            uv = u[:].rearrange("p (j c) -> p j c", j=nj)
            nc.scalar.activation(e2t[:, :nj, :], uv, Act.Exp, scale=1.0)
            nc.scalar.activation(
                e2t[:, nj:, :], uv, Act.Exp, scale=-1.0, bias=bigbias[:]
            )

            # tree over the 10 c-planes: (0-4)+(5-9); {0+2,1+3}; +; + plane 4
            ta = work.tile([P, 5 * jj], bf16, tag="ta")
            nc.vector.tensor_tensor(ta[:], e2[:, : 5 * jj], e2[:, 5 * jj :], op=Alu.add)
            tb = work.tile([P, 2 * jj], bf16, tag="tb")
            nc.vector.tensor_tensor(tb[:], ta[:, : 2 * jj], ta[:, 2 * jj : 4 * jj], op=Alu.add)
            tc = work.tile([P, jj], bf16, tag="tc")
            nc.vector.tensor_tensor(tc[:], tb[:, :jj], tb[:, jj:], op=Alu.add)
            s2 = work.tile([P, jj], f32, tag="s2")
            nc.vector.tensor_tensor(s2[:], tc[:], ta[:, 4 * jj :], op=Alu.add)

            nc.vector.tensor_tensor(
                prodbuf[:, v * NJ + j0 : v * NJ + j0 + nj],
                s2[:, :nj], s2[:, nj:], op=Alu.mult,
            )

        # epilogue, split so only the last section's chunk sits on the
        # critical path after the final reduce
        nsplit = (V - 1) * NJ
        termbuf = acc.tile([P, V * NJ], f32)
        nc.scalar.activation(
            termbuf[:, :nsplit], prodbuf[:, :nsplit], Act.Ln, bias=1.0, scale=1.0
        )
        nc.scalar.activation(
            termbuf[:, nsplit:], prodbuf[:, nsplit:], Act.Ln, bias=1.0, scale=1.0
        )
        colsum2 = acc.tile([P, 2], f32)
        nc.vector.tensor_reduce(
            colsum2[:, 0:1], termbuf[:, :nsplit].unsqueeze(1), axis=Axis.X, op=Alu.add
        )
        nc.vector.tensor_reduce(
            colsum2[:, 1:2], termbuf[:, nsplit:].unsqueeze(1), axis=Axis.X, op=Alu.add
        )
        # partition-sum on gpsimd so the output DMA is a single 4-byte
        # descriptor (a [128,1] store costs ~8us in descriptor/receipt time)
        total = acc.tile([1, 1], f32)
        nc.gpsimd.tensor_reduce(total[:], colsum2[:], axis=Axis.XYZWC, op=Alu.add)
        nc.sync.dma_start(out_d, total[:])

    nc.compile()
    return nc


_PROGRAM_CACHE = {}


def _get_program():
    key = (V, NJ)
    if key not in _PROGRAM_CACHE:
        nc = bacc.Bacc("TRN2", target_bir_lowering=False, debug=False)
        build_core_program(nc)
        _PROGRAM_CACHE[key] = nc
    return _PROGRAM_CACHE[key]


def kernel(T, bayes, partial, _trace=False):
    assert T.shape == (B, C, C) and bayes.shape == (B,) and partial.shape == (B, C)
    import ml_dtypes

    f8np = ml_dtypes.float8_e3m4

    bay = np.asarray(bayes).astype(np.int64)

    # fp8 rows: T[i] flattened r-major, with -BIG*partial[i] written into the
    # dead row (bayes[i]+1)%C of each element's block (never read for that
    # element). Rows B..B+C-1 are per-section null elements: T=0, mask=-BIG
    # in the section's dead row.
    T8 = np.empty((B + V, CC), f8np)
    T8[:B] = np.asarray(T, np.float32).reshape(B, CC).astype(f8np)
    m8 = (np.asarray(partial).astype(np.float32) * (-BIG)).astype(f8np)
    w = ((bay + 1) % V) * C
    cols = w[:, None] + np.arange(C)[None, :]
    np.put_along_axis(T8[:B], cols, m8, axis=1)
    T8[B:] = 0.0
    for v in range(V):
        T8[B + v, ((v + 1) % V) * C : ((v + 1) % V) * C + C] = -BIG

    order = np.argsort(bay, kind="stable")
    counts = np.bincount(bay, minlength=V)
    assert len(counts) == V

    perms = np.full((NCORES, S_CORE), B, dtype=np.int64)
    start = 0
    for v in range(V):
        perms[:, v * S_V : (v + 1) * S_V] = B + v
        bucket = order[start : start + counts[v]]
        start += counts[v]
        for k in range(NCORES):
            sub = bucket[k::NCORES]
            assert len(sub) <= S_V, f"bucket overflow v={v} core={k}: {len(sub)}"
            perms[k, v * S_V : v * S_V + len(sub)] = sub

    in_maps = [{"t_in": T8[perms[k]]} for k in range(NCORES)]

    nc = _get_program()
    res = run_bass_kernel_spmd(
        nc, in_maps, core_ids=list(range(NCORES)), trace=_trace
    )
    total = sum(float(res.results[k]["sum_out"][0, 0]) for k in range(NCORES))
    out = np.float32(total / B)
    if _trace:
        return out, res
    return out


# revision 36
# speedup vs baseline: 2.0925x; 2.0925x over previous
"""LSEP loss kernel for Trainium2, data-parallel over 8 NeuronCores.

Math per element i (B=1e6, C=10):
  q[c]  = T[i, bayes[i], c]
  s_neg = sum_c (partial[i,c]==0) * exp(q[c])
  s_pos = sum_c (partial[i,c]==1) * exp(-q[c])
  loss  = mean_i log1p(s_neg * s_pos)

Strategy: the loss is a mean, so elements can be freely permuted. Host-side
we bucket elements by bayes value and give every core a static layout of
10 sections (one per bayes value v), each 128 partitions x 100 slots, padded
with null elements (T=0, partial=1 -> contributes ~2e-7 rel, negligible).
Row selection then needs no gather: section v reads T columns [v*10, v*10+10)
through a static strided access pattern, so no compute engine touches the 90
unused T values -- they only flow through DMA, which is the intended
memory-bound term. T is staged as fp8(e3m4) rows of 100 bytes.

The partial mask is folded ADDITIVELY: the host writes m = -15.5*p as fp8
(both values exact; 15.5 is the e3m4 max normal) into the DEAD row
(v+1)%10 of each element's T block -- that row is never read for a
section-v element, so the mask rides along in bytes that were already
being transferred. The device computes u = q + m in one gpsimd add
(bf16 out, exact for p=0 since q is fp8).
Then  s_neg = sum_c exp(u)        (masked terms are scaled by e^-15.5)
      s_pos = sum_c exp(-u - 15.5)  (ACT's free scale/bias: func(s*x+b))
so each section is exactly: 1 gpsimd add, 2 ACT exps (flat bf16), 2 DVE
innermost-reduces, 1 small product. No mask multiplies, no DVE two-port ops
(which lock gpsimd out of SBUF), minimal SBUF traffic: under concurrent DMA
streaming every engine's effective rate drops ~2x from port contention, so
the op mix matters more than isolated op speed. Verified numerics vs the
f32 reference: rel err 2.6e-4 (gate 2e-2). Epilogue: log1p, row-sum, one
[128,1] output per core; host sums across cores/partitions, divides by B.
"""

from contextlib import ExitStack

import numpy as np

import concourse.bacc as bacc
import concourse.mybir as mybir
import concourse.tile as tile
from concourse.bass_utils import run_bass_kernel_spmd

f32 = mybir.dt.float32
bf16 = mybir.dt.bfloat16
f8 = mybir.dt.float8e3
Alu = mybir.AluOpType
Act = mybir.ActivationFunctionType
Axis = mybir.AxisListType

B = 1_000_000
C = 10
CC = C * C
NCORES = 8
V = C  # bayes values / sections
P = 128
NJ = 100  # slots per partition per section
S_V = P * NJ  # 12800 slots per (core, section)
S_CORE = V * S_V  # 128000 slots per core
BIG = 15.5  # e3m4 max normal; exp(-BIG) ~ 1.9e-7 suppresses masked terms
assert NCORES * S_V >= B // V + 8 * 300  # ~8 sigma headroom per bucket


def build_core_program(nc):
    T_d = nc.dram_tensor("t_in", [S_CORE, CC], f8, kind="ExternalInput").ap()
    out_d = nc.dram_tensor("sum_out", [1, 1], f32, kind="ExternalOutput").ap()

    T_view = T_d.rearrange("(v p j) c -> v p (j c)", v=V, p=P, j=NJ)

    # full tiles up front (transfer > descriptor-gen keeps the DMA ring fed;
    # small leading tiles starve it), split tiles at the end to shorten the
    # serial tail chain after the last DMA lands
    q = NJ // 4
    subtiles = [(v, 0, NJ) for v in range(V - 1)]
    subtiles += [(V - 1, 0, 2 * q), (V - 1, 2 * q, q), (V - 1, 3 * q, q)]

    with tile.TileContext(nc) as tc, ExitStack() as ctx:
        big = ctx.enter_context(tc.tile_pool(name="big", bufs=4))
        work = ctx.enter_context(tc.tile_pool(name="work", bufs=3))
        acc = ctx.enter_context(tc.tile_pool(name="acc", bufs=1))

        prodbuf = acc.tile([P, V * NJ], f32)
        bigbias = acc.tile([P, 1], f32)
        nc.vector.memset(bigbias[:], -BIG)

        for v, j0, nj in subtiles:
            t = big.tile([P, nj * CC], f8, tag="t")
            nc.sync.dma_start(t[:], T_view[v, :, j0 * CC : (j0 + nj) * CC])
            tv = t[:].rearrange("p (j c) -> p j c", j=nj)
            tsel = tv[:, :, v * C : (v + 1) * C]  # [P, nj, 10]: T row v
            # mask -BIG*p lives in the dead row (v+1)%V of each element's
            # T block (never read for a section-v element)
            w = (v + 1) % V
            pm = tv[:, :, w * C : (w + 1) * C]
            n = nj * C

            # u = q - BIG*p in one gpsimd add (bf16; exact where p=0)
            u = work.tile([P, n], bf16, tag="u")
            nc.gpsimd.tensor_tensor(
                u[:].rearrange("p (j c) -> p j c", j=nj), tsel, pm, op=Alu.add
            )

            # both exps land in one tile so a single innermost-reduce yields
            # s_neg (rows 0..nj) and s_pos (rows nj..2nj) in one instruction
            e2 = work.tile([P, 2 * n], bf16, tag="e2")
            nc.scalar.activation(e2[:, :n], u[:], Act.Exp, scale=1.0)
            nc.scalar.activation(e2[:, n:], u[:], Act.Exp, scale=-1.0, bias=bigbias[:])

            s2 = work.tile([P, 2 * nj], f32, tag="s2")
            nc.vector.tensor_reduce(
                s2[:], e2[:].rearrange("p (j c) -> p j c", j=2 * nj),
                axis=Axis.X, op=Alu.add,
            )

            nc.vector.tensor_tensor(
                prodbuf[:, v * NJ + j0 : v * NJ + j0 + nj],
                s2[:, :nj], s2[:, nj:], op=Alu.mult,
            )

        # epilogue, split so only the last section's chunk sits on the
        # critical path after the final reduce
        nsplit = (V - 1) * NJ
        termbuf = acc.tile([P, V * NJ], f32)
        nc.scalar.activation(
            termbuf[:, :nsplit], prodbuf[:, :nsplit], Act.Ln, bias=1.0, scale=1.0
        )
        nc.scalar.activation(
            termbuf[:, nsplit:], prodbuf[:, nsplit:], Act.Ln, bias=1.0, scale=1.0
        )
        colsum2 = acc.tile([P, 2], f32)
        nc.vector.tensor_reduce(
            colsum2[:, 0:1], termbuf[:, :nsplit].unsqueeze(1), axis=Axis.X, op=Alu.add
        )
        nc.vector.tensor_reduce(
            colsum2[:, 1:2], termbuf[:, nsplit:].unsqueeze(1), axis=Axis.X, op=Alu.add
        )
        # partition-sum on gpsimd so the output DMA is a single 4-byte
        # descriptor (a [128,1] store costs ~8us in descriptor/receipt time)
        total = acc.tile([1, 1], f32)
        nc.gpsimd.tensor_reduce(total[:], colsum2[:], axis=Axis.XYZWC, op=Alu.add)
        nc.sync.dma_start(out_d, total[:])

    nc.compile()
    return nc


_PROGRAM_CACHE = {}


def _get_program():
    key = (V, NJ)
    if key not in _PROGRAM_CACHE:
        nc = bacc.Bacc("TRN2", target_bir_lowering=False, debug=False)
        build_core_program(nc)
        _PROGRAM_CACHE[key] = nc
    return _PROGRAM_CACHE[key]


def kernel(T, bayes, partial, _trace=False):
    assert T.shape == (B, C, C) and bayes.shape == (B,) and partial.shape == (B, C)
    import ml_dtypes

    f8np = ml_dtypes.float8_e3m4

    bay = np.asarray(bayes).astype(np.int64)

    # fp8 rows: T[i] flattened r-major, with -BIG*partial[i] written into the
    # dead row (bayes[i]+1)%C of each element's block (never read for that
    # element). Rows B..B+C-1 are per-section null elements: T=0, mask=-BIG
    # in the section's dead row.
    T8 = np.empty((B + V, CC), f8np)
    T8[:B] = np.asarray(T, np.float32).reshape(B, CC).astype(f8np)
    m8 = (np.asarray(partial).astype(np.float32) * (-BIG)).astype(f8np)
    w = ((bay + 1) % V) * C
    cols = w[:, None] + np.arange(C)[None, :]
    np.put_along_axis(T8[:B], cols, m8, axis=1)
    T8[B:] = 0.0
    for v in range(V):
        T8[B + v, ((v + 1) % V) * C : ((v + 1) % V) * C + C] = -BIG

    order = np.argsort(bay, kind="stable")
    counts = np.bincount(bay, minlength=V)
    assert len(counts) == V

    perms = np.full((NCORES, S_CORE), B, dtype=np.int64)
    start = 0
    for v in range(V):
        perms[:, v * S_V : (v + 1) * S_V] = B + v
        bucket = order[start : start + counts[v]]
        start += counts[v]
        for k in range(NCORES):
            sub = bucket[k::NCORES]
            assert len(sub) <= S_V, f"bucket overflow v={v} core={k}: {len(sub)}"
            perms[k, v * S_V : v * S_V + len(sub)] = sub

    in_maps = [{"t_in": T8[perms[k]]} for k in range(NCORES)]

    nc = _get_program()
    res = run_bass_kernel_spmd(
        nc, in_maps, core_ids=list(range(NCORES)), trace=_trace
    )
    total = sum(float(res.results[k]["sum_out"][0, 0]) for k in range(NCORES))
    out = np.float32(total / B)
    if _trace:
        return out, res
    return out
